# revision 15
# baseline (speedup 1.0000x reference)
"""Two-layer GCN (DGL GraphConv norm='both') on 8 Trainium2 NeuronCores.

Strategy
--------
Both layers are  out = A_norm @ X @ W + b  with the same normalized adjacency
A_norm = D_in^-1/2 A D_out^-1/2 (1.6M edges over 100k nodes).  All index-only
math (degrees, rsqrt norms, per-edge weight w_e = ns[src]*nd[dst], edge
partitioning/sorting) happens on the host.

Nodes are partitioned contiguously across the 8 cores (12544 = 98 tiles of
128 rows each).  Edges live with their dst core, sorted by (dst tile, src
chunk).  Per 128-edge block the device:
  - dma_gather's the 128 source rows (fp16, 256B each) from a replicated
    node-feature table (int16 gather indices => the table is split in 4
    chunks of 25088 rows),
  - builds a routing matrix M[e, d] = (iota[d] == rank_e) * w_e with one
    fused tensor_scalar op,
  - accumulates psum[f, d] += G_block.T @ M_block on the TensorEngine (fp16
    in, fp32 accumulate).
Per dst tile the aggregated [feat, dst] psum is then multiplied by W (fp32)
and relu'd (layer 1, output cast to fp16 for the next layer's gather table).
Between layers a single AllGather shares the h1 shards.  b2 is added on the
host (pure post-add); b1 is folded in on device only if nonzero.
"""

import numpy as np

for _p in ("/opt/trn_rl_repo",):
    import sys
    if _p not in sys.path:
        sys.path.insert(0, _p)

from concourse import bacc, bass, mybir
import concourse.tile as tile
from concourse.bass_utils import run_bass_kernel_spmd

# problem constants (hardcoded per harness contract)
N_NODES = 100000
N_EDGES = 1600000
FIN = 128
HID = 128
NCLS = 64

NCORE = 8
P = 128
TILES_PER_CORE = 98
NSHARD = TILES_PER_CORE * P          # 12544
NPAD = NCORE * NSHARD                # 100352
NCH = 4
CHUNK = NPAD // NCH                  # 25088, int16-safe gather chunk
G_TILES = 7                          # dst tiles per gather group
NGROUP = TILES_PER_CORE // G_TILES   # 14


def _set_dims(n_nodes, n_edges, tiles_per_core, g_tiles):
    """Debug hook: downscale the problem (defaults match the harness)."""
    global N_NODES, N_EDGES, TILES_PER_CORE, NSHARD, NPAD, CHUNK, G_TILES, NGROUP
    N_NODES, N_EDGES = n_nodes, n_edges
    TILES_PER_CORE = tiles_per_core
    NSHARD = TILES_PER_CORE * P
    NPAD = NCORE * NSHARD
    assert NPAD % NCH == 0 and NPAD // NCH <= 32768
    CHUNK = NPAD // NCH
    G_TILES = g_tiles
    NGROUP = TILES_PER_CORE // G_TILES
    assert NGROUP * G_TILES == TILES_PER_CORE

TRACE = False                        # test harness flips this for profiling
_LAST_RESULTS = {}                   # exec_time etc. for the test harness


def _pack_idx(flat: np.ndarray) -> np.ndarray:
    """dma_gather idx layout: idx j at [j%16 + 16g, j//16], replicated to the
    8 GpSimd core groups."""
    n = len(flat)
    assert n % 16 == 0
    return np.tile(flat.reshape(n // 16, 16).T, (8, 1)).astype(np.int16)


def _preprocess(src, dst, w_edge):
    """Host-side edge layout. Returns the (core-independent) block structure
    plus per-core index/metadata arrays."""
    src = src.astype(np.int64)
    dst = dst.astype(np.int64)

    tile_g = dst >> 7
    core_of = tile_g // TILES_PER_CORE
    tloc = tile_g % TILES_PER_CORE
    ch = src // CHUNK
    cell = tloc * NCH + ch                       # 0..391
    NCELL = TILES_PER_CORE * NCH

    counts = np.zeros((NCORE, NCELL), np.int64)
    for c in range(NCORE):
        counts[c] = np.bincount(cell[core_of == c], minlength=NCELL)
    nb_cell = -(-counts.max(axis=0) // P)        # blocks per (tile, chunk)
    nb_cell = nb_cell.reshape(TILES_PER_CORE, NCH)

    # global block/column enumeration: groups -> chunks -> tiles -> blocks.
    # A single dma_gather call is capped at MAXBLK blocks (descriptor-ring
    # headroom: 32 blocks = 4096 descs = 256/engine, ring holds 512/engine).
    import os as _os1
    MAXBLK = int(_os1.environ.get("KMAXBLK", "8"))
    col0_cell = np.zeros((TILES_PER_CORE, NCH), np.int64)
    calls = []                                   # (g, ch, col0, nblocks)
    col = 0
    for g in range(NGROUP):
        ts0 = g * G_TILES
        for c_h in range(NCH):
            c0 = col
            for t in range(ts0, ts0 + G_TILES):
                col0_cell[t, c_h] = col
                col += nb_cell[t, c_h]
            for s in range(c0, col, MAXBLK):
                calls.append((g, c_h, s, min(MAXBLK, col - s)))
    nblk = col

    # per-tile matmul block columns (same for every core)
    tile_cols = []
    for t in range(TILES_PER_CORE):
        cols = np.concatenate(
            [col0_cell[t, c_h] + np.arange(nb_cell[t, c_h]) for c_h in range(NCH)]
        ) if nb_cell[t].sum() else np.empty(0, np.int64)
        tile_cols.append(cols)

    per_core = []
    for c in range(NCORE):
        m = core_of == c
        cell_c = cell[m]
        order = np.argsort(cell_c, kind="stable")
        cell_s = cell_c[order]
        src_s = (src[m][order] % CHUNK).astype(np.int16)
        rank_s = (dst[m][order] & 127).astype(np.float32)
        w_s = w_edge[m][order].astype(np.float32)

        starts = np.zeros(NCELL + 1, np.int64)
        starts[1:] = np.cumsum(np.bincount(cell_s, minlength=NCELL))
        q = np.arange(len(cell_s)) - starts[cell_s]
        colE = col0_cell.reshape(-1)[cell_s] + (q >> 7)
        pE = q & 127

        rank_arr = np.zeros((P, nblk), np.float32)
        w_arr = np.zeros((P, nblk), np.float32)
        idx_flat = np.zeros(nblk * P, np.int16)
        rank_arr[pE, colE] = rank_s
        w_arr[pE, colE] = w_s
        idx_flat[colE * P + pE] = src_s
        per_core.append((rank_arr, w_arr, _pack_idx(idx_flat)))

    return nb_cell, calls, tile_cols, nblk, per_core


def _build_program(calls, tile_cols, nblk, need_b1):
    f16, f32 = mybir.dt.float16, mybir.dt.float32
    # default 16KB descriptor carveout = 1024 descs -> a single dma_gather
    # call must stay <= 8 blocks (1024 indices)
    nc = bacc.Bacc(None, num_devices=NCORE)

    xh_d = nc.declare_dram_parameter("xh", [NPAD, FIN], f16, isOutput=False)
    W1_d = nc.declare_dram_parameter("W1", [FIN, HID], f32, isOutput=False)
    W2_d = nc.declare_dram_parameter("W2", [HID, NCLS], f32, isOutput=False)
    if need_b1:
        b1_d = nc.declare_dram_parameter("b1", [1, HID], f32, isOutput=False)
    rank_d = nc.declare_dram_parameter("rank", [P, nblk], f32, isOutput=False)
    wgt_d = nc.declare_dram_parameter("wgt", [P, nblk], f32, isOutput=False)
    idx_d = nc.declare_dram_parameter("gidx", [P, nblk * 8], mybir.dt.int16,
                                      isOutput=False)
    out_d = nc.declare_dram_parameter("out", [NSHARD, NCLS], f32, isOutput=True)

    h1_own = nc.dram_tensor("h1_own", [NSHARD, HID], f16)
    h1_full = nc.dram_tensor("h1_full", [NPAD, HID], f16, addr_space="Shared")
    import os as _os0
    if _os0.environ.get("KXH_INTERNAL") == "1":
        xh_int = nc.dram_tensor("xh_int", [NPAD, FIN], f16)
    else:
        xh_int = None

    # group -> list of (tile, [block cols]) and per-group col ranges
    grp_tiles = []
    grp_c0 = []
    grp_nb = []
    for g in range(NGROUP):
        ts0 = g * G_TILES
        tl = [(t, tile_cols[t]) for t in range(ts0, ts0 + G_TILES)]
        cols_all = np.concatenate([c for _, c in tl if len(c)])
        grp_tiles.append(tl)
        grp_c0.append(int(cols_all.min()))
        grp_nb.append(int(cols_all.max()) - int(cols_all.min()) + 1)
    max_gnb = max(grp_nb)

    with tile.TileContext(nc) as tc:
        with (
            tc.tile_pool(name="const", bufs=1) as cp,
            tc.tile_pool(name="gpool", bufs=2) as gp,
            tc.tile_pool(name="ipool", bufs=4) as ip,
            tc.tile_pool(name="mpool", bufs=8) as mp,
            tc.tile_pool(name="apool", bufs=3) as ap_,
            tc.tile_pool(name="hpool", bufs=3) as hp_,
            tc.tile_pool(name="psum_a", bufs=4, space="PSUM") as ppa,
            tc.tile_pool(name="psum_h", bufs=2, space="PSUM") as pph,
        ):
            rank_t = cp.tile([P, nblk], f32)
            wgt_t = cp.tile([P, nblk], f32)
            W1_t = cp.tile([FIN, HID], f32)
            W2_t = cp.tile([HID, NCLS], f32)
            nc.sync.dma_start(rank_t[:], rank_d[:])
            nc.sync.dma_start(wgt_t[:], wgt_d[:])
            nc.sync.dma_start(W1_t[:], W1_d[:])
            nc.sync.dma_start(W2_t[:], W2_d[:])

            iota_t = cp.tile([P, P], f16)
            nc.gpsimd.iota(iota_t[:], pattern=[[1, P]], base=0,
                           channel_multiplier=0,
                           allow_small_or_imprecise_dtypes=True)

            if need_b1:
                b1row = cp.tile([1, HID], f32)
                ones1 = cp.tile([1, P], f32)
                nc.sync.dma_start(b1row[:], b1_d[:])
                nc.gpsimd.memset(ones1[:], 1.0)
                b1_ps = pph.tile([P, HID], f32)
                nc.tensor.matmul(out=b1_ps[:], lhsT=ones1[:], rhs=b1row[:],
                                 start=True, stop=True)
                b1_bc = cp.tile([P, HID], f32)
                nc.vector.tensor_copy(b1_bc[:], b1_ps[:])

            # warm DVE's observed clock on one-time producers so each
            # tensor_scalar below needs at most one hw sync-wait slot
            scr = cp.tile([P, 4], f32)
            nc.vector.tensor_copy(scr[:, 0:1], rank_t[:, 0:1])
            nc.vector.tensor_copy(scr[:, 1:2], wgt_t[:, 0:1])
            nc.vector.tensor_copy(scr[:, 2:3],
                                  iota_t[:, 0:2].bitcast(f32)[:, 0:1])

            import os as _os
            _skip_ag = _os.environ.get("KSKIP_AG") == "1"
            _l2_xh = _os.environ.get("KL2_SRC") == "xh"
            _xh_src = xh_d
            if xh_int is not None:
                nc.gpsimd.dma_start(out=xh_int[:], in_=xh_d[:])
                _xh_src = xh_int
            _layers = (1,) if _os.environ.get("KONLY_L1") == "1" else (1, 2)
            for layer in _layers:
                table = _xh_src if (layer == 1 or _l2_xh) else h1_full
                W_t = W1_t if layer == 1 else W2_t
                ncol = HID if layer == 1 else NCLS

                for g in range(NGROUP):
                    c0g, nbg = grp_c0[g], grp_nb[g]
                    g_t = gp.tile([P, max_gnb, FIN], f16, tag="G")
                    if _os.environ.get("KNO_GATHER") == "1":
                        nc.gpsimd.memset(g_t[:, 0:1, :], 0.0)
                    for (gg, c_h, ccol0, cnb) in calls:
                        if gg != g or _os.environ.get("KNO_GATHER") == "1":
                            continue
                        nidx = cnb * P
                        idx_ct = ip.tile([P, cnb * 8], mybir.dt.int16,
                                         tag="idxcall")
                        nc.sync.dma_start(idx_ct[:],
                                          idx_d[:, ccol0 * 8:(ccol0 + cnb) * 8])
                        nc.gpsimd.dma_gather(
                            out_ap=g_t[:, ccol0 - c0g:ccol0 - c0g + cnb, :],
                            in_ap=table[c_h * CHUNK:(c_h + 1) * CHUNK, :],
                            idxs_ap=idx_ct[:],
                            num_idxs=nidx,
                            num_idxs_reg=nidx,
                            elem_size=FIN,
                        )

                    for (t, cols) in grp_tiles[g]:
                        rows = slice(t * P, (t + 1) * P)
                        if len(cols) == 0:
                            zt = hp_.tile([P, ncol], f16 if layer == 1 else f32,
                                          tag="zero")
                            nc.gpsimd.memset(zt[:], 0.0)
                            nc.sync.dma_start(
                                (h1_own if layer == 1 else out_d)[rows, :], zt[:])
                            continue
                        agg_ps = ppa.tile([FIN, P], f32)
                        for i, c in enumerate(cols):
                            c = int(c)
                            m_t = mp.tile([P, P], f16, tag="M")
                            nc.vector.tensor_scalar(
                                out=m_t[:], in0=iota_t[:],
                                scalar1=rank_t[:, c:c + 1],
                                scalar2=wgt_t[:, c:c + 1],
                                op0=mybir.AluOpType.is_equal,
                                op1=mybir.AluOpType.mult,
                            )
                            nc.tensor.matmul(
                                out=agg_ps[:], lhsT=g_t[:, c - c0g, :], rhs=m_t[:],
                                start=(i == 0), stop=(i == len(cols) - 1),
                            )
                        aggT_s = ap_.tile([FIN, P], f32, tag="aggT")
                        nc.vector.tensor_copy(aggT_s[:], agg_ps[:])
                        h_ps = pph.tile([P, ncol], f32, tag="hps")
                        nc.tensor.matmul(out=h_ps[:], lhsT=aggT_s[:],
                                         rhs=W_t[:, :ncol], start=True, stop=True)
                        if layer == 1:
                            if need_b1:
                                nc.vector.tensor_tensor(
                                    out=h_ps[:], in0=h_ps[:], in1=b1_bc[:],
                                    op=mybir.AluOpType.add)
                            h_s = hp_.tile([P, HID], f16, tag="h1")
                            nc.scalar.activation(
                                h_s[:], h_ps[:], mybir.ActivationFunctionType.Relu)
                            nc.sync.dma_start(h1_own[rows, :], h_s[:])
                        else:
                            o_s = hp_.tile([P, NCLS], f32, tag="out")
                            nc.scalar.copy(o_s[:], h_ps[:])
                            nc.sync.dma_start(out_d[rows, :], o_s[:])

                if layer == 1 and not _skip_ag:
                    nc.gpsimd.collective_compute(
                        "AllGather",
                        mybir.AluOpType.bypass,
                        replica_groups=[list(range(NCORE))],
                        ins=[h1_own[:]],
                        outs=[h1_full[:]],
                    )

    nc.finalize()
    return nc


def kernel(inputs, src, dst, W1, b1, W2, b2):
    inputs = np.asarray(inputs, dtype=np.float32)
    src_i = np.asarray(src, dtype=np.int64)
    dst_i = np.asarray(dst, dtype=np.int64)
    W1 = np.asarray(W1, dtype=np.float32)
    b1 = np.asarray(b1, dtype=np.float32)
    W2 = np.asarray(W2, dtype=np.float32)
    b2 = np.asarray(b2, dtype=np.float32)

    # degree norms (matches jax segment_sum/clip/rsqrt in fp32)
    deg_out = np.bincount(src_i, minlength=N_NODES).astype(np.float32)
    deg_in = np.bincount(dst_i, minlength=N_NODES).astype(np.float32)
    ns = (1.0 / np.sqrt(np.maximum(deg_out, 1.0))).astype(np.float32)
    nd = (1.0 / np.sqrt(np.maximum(deg_in, 1.0))).astype(np.float32)
    w_edge = (ns[src_i] * nd[dst_i]).astype(np.float32)

    nb_cell, calls, tile_cols, nblk, per_core = _preprocess(src_i, dst_i, w_edge)

    xh = np.zeros((NPAD, FIN), np.float16)
    xh[:N_NODES] = inputs.astype(np.float16)

    need_b1 = bool(np.any(b1 != 0))
    nc = _build_program(calls, tile_cols, nblk, need_b1)

    in_maps = []
    for c in range(NCORE):
        rank_arr, w_arr, idx_packed = per_core[c]
        m = {
            "xh": xh,
            "W1": W1,
            "W2": W2,
            "rank": rank_arr,
            "wgt": w_arr,
            "gidx": idx_packed.reshape(P, nblk * 8),
        }
        if need_b1:
            m["b1"] = b1.reshape(1, HID)
        in_maps.append(m)

    res = run_bass_kernel_spmd(nc, in_maps, list(range(NCORE)), trace=TRACE)
    _LAST_RESULTS["exec_time_ns"] = res.exec_time_ns
    _LAST_RESULTS["res"] = res

    out = np.concatenate([res.results[c]["out"] for c in range(NCORE)], axis=0)
    out = out[:N_NODES].astype(np.float32)
    if np.any(b2 != 0):
        out = out + b2[None, :]
    return out
